# revision 1
# baseline (speedup 1.0000x reference)
"""Causal GQA self-attention (B=2, S=2048, H=2048, 16 q-heads / 4 kv-heads,
head_dim=128, RoPE) as a Bass/Tile kernel on 8 TRN2 NeuronCores.

Sharding: tensor-parallel over heads. Core c owns q-heads {2c, 2c+1} and
kv-head c//2; it computes a full [B, S, H] partial of the output projection
(o_partial = attn_out_c @ wo_c) and the host sums the 8 partials.

On-chip layout notes:
 - x is pre-transposed on the host to xT [B, H, S] so the H contraction of
   the q/k/v projections lands on the partition dim.
 - q/k are produced transposed ([head_dim, S]) straight out of the PE
   (lhsT = weight tile, rhs = xT tile). RoPE's rotate-half is a partition
   permutation, done as a tiny extra PE matmul against a constant
   signed-permutation matrix; the elementwise cos/sin work runs on DVE.
 - v is transposed back to natural layout with PE transposes (identity
   matmul) — no DMA-xbar transposes and no SBUF->SBUF DMAs anywhere.
 - scores are computed transposed (sT[kj, qi] = kT_j^T . qT) so softmax's
   exp reads PE-fresh PSUM; softmax runs max-free (scores are ~N(0,1), exp
   cannot overflow) with the denominator accumulated by an all-ones
   [128,128] matmul into PSUM alongside the PV accumulation.
 - PV uses v natural layout as the stationary operand: outT[d, qi] += v_j^T
   . pT, which leaves the attention output already transposed for the
   o-projection.
 - matmul dtype is float32r (TF32-ish, full PE rate at N>=256) for the
   projections / o-proj, bf16 for q/k/p/v in the attention core.
"""

import math

import numpy as np
import ml_dtypes

import concourse.bass as bass
import concourse.tile as tile
from concourse import mybir
from concourse.bass_utils import run_bass_kernel_spmd

F32 = mybir.dt.float32
F32R = mybir.dt.float32r
BF16 = mybir.dt.bfloat16
AF = mybir.ActivationFunctionType

B, S, H = 2, 2048, 2048
NH, NKV, HD = 16, 4, 128
N_CORES = 8
KT = H // 128          # 16 k-tiles over the H contraction
SC = 512               # proj s-chunk width
NSC = S // SC          # 4
QT = 1024              # attention qi tile width
NQT = S // QT          # 2
SCALE = 1.0 / math.sqrt(HD)
ROPE_BASE = 10000.0


def _alu(name):
    from concourse.alu_op_type import AluOpType

    return getattr(AluOpType, name)


def legalize_waits(nc, cap=1):
    """walrus in this container accepts at most one sync-wait per
    instruction; move excess waits onto NoOp carriers just before the
    instruction on the same engine (sequencers run waits in order, so this
    is semantically identical)."""
    n_split = 0
    for f in nc.m.functions:
        for blk in f.blocks:
            if not any(
                i.sync_info is not None and len(i.sync_info.on_wait) > cap
                for i in blk.instructions
            ):
                continue
            new_insts = []
            for inst in blk.instructions:
                si = inst.sync_info
                waits = list(si.on_wait) if si is not None else []
                if len(waits) > cap:
                    for k, w in enumerate(waits[:-cap]):
                        new_insts.append(
                            mybir.InstNoOp(
                                name=f"{inst.name}-wsplit{k}",
                                engine=inst.engine,
                                sync_info=mybir.SyncInfo(on_wait=[w], on_update=[]),
                            )
                        )
                        n_split += 1
                    inst.sync_info = mybir.SyncInfo(
                        on_wait=waits[-cap:], on_update=list(si.on_update)
                    )
                new_insts.append(inst)
            blk.instructions = new_insts
    return n_split


def build_nc():
    mult = _alu("mult")
    add = _alu("add")

    nc = bass.Bass(trn_type="TRN2", target_bir_lowering=False)

    xT_d = nc.dram_tensor("xT", [B, H, S], F32R, kind="ExternalInput")
    wq_d = nc.dram_tensor("wq", [H, 2 * HD], F32R, kind="ExternalInput")
    wk_d = nc.dram_tensor("wk", [H, HD], F32R, kind="ExternalInput")
    wv_d = nc.dram_tensor("wv", [H, HD], F32R, kind="ExternalInput")
    wo_d = nc.dram_tensor("wo", [2 * HD, H], F32R, kind="ExternalInput")
    cos_d = nc.dram_tensor("cosT", [HD, S], BF16, kind="ExternalInput")
    sinrot_d = nc.dram_tensor("sinrotT", [HD, S], F32, kind="ExternalInput")
    mask_d = nc.dram_tensor("addmask", [128, 128], F32, kind="ExternalInput")
    rotm_d = nc.dram_tensor("rotmT", [128, 128], BF16, kind="ExternalInput")
    iden_d = nc.dram_tensor("iden", [128, 128], BF16, kind="ExternalInput")
    o_d = nc.dram_tensor("o", [B, S, H], F32R, kind="ExternalOutput")

    with tile.TileContext(nc) as tc:
        with (
            tc.tile_pool(name="consts", bufs=1) as consts,
            tc.tile_pool(name="xpool", bufs=2) as xpool,
            tc.tile_pool(name="homes", bufs=1) as homes,
            tc.tile_pool(name="stage", bufs=3) as stage,
            tc.tile_pool(name="ptp", bufs=3) as ptp,
            tc.tile_pool(name="epi", bufs=2) as epi,
            tc.tile_pool(name="opool", bufs=3) as opool,
            tc.tile_pool(name="ps", bufs=4, space="PSUM") as ps,
        ):
            # ---- constants ----
            wq_sb = consts.tile([128, KT, 2 * HD], F32R, tag="wq_sb")
            nc.sync.dma_start(out=wq_sb, in_=wq_d.ap().rearrange("(k p) d -> p k d", p=128))
            wk_sb = consts.tile([128, KT, HD], F32R, tag="wk_sb")
            nc.sync.dma_start(out=wk_sb, in_=wk_d.ap().rearrange("(k p) d -> p k d", p=128))
            wv_sb = consts.tile([128, KT, HD], F32R, tag="wv_sb")
            nc.sync.dma_start(out=wv_sb, in_=wv_d.ap().rearrange("(k p) d -> p k d", p=128))
            wo_sb = consts.tile([128, 2, H], F32R, tag="wo_sb")
            nc.sync.dma_start(out=wo_sb, in_=wo_d.ap().rearrange("(c p) n -> p c n", p=128))
            cos_sb = consts.tile([HD, S], BF16, tag="cos_sb")
            nc.sync.dma_start(out=cos_sb, in_=cos_d.ap())
            sinrot_sb = consts.tile([HD, S], F32, tag="sinrot_sb")
            nc.sync.dma_start(out=sinrot_sb, in_=sinrot_d.ap())
            mask_sb = consts.tile([128, 128], F32, tag="mask_sb")
            nc.sync.dma_start(out=mask_sb, in_=mask_d.ap())
            rotm_sb = consts.tile([128, 128], BF16, tag="rotm_sb")
            nc.sync.dma_start(out=rotm_sb, in_=rotm_d.ap())
            iden_sb = consts.tile([128, 128], BF16, tag="iden_sb")
            nc.sync.dma_start(out=iden_sb, in_=iden_d.ap())
            ones_sb = consts.tile([128, 128], BF16, tag="ones_sb")
            nc.vector.memset(ones_sb, 1.0)

            for b in range(B):
                # ---- per-batch homes ----
                q0_sb = homes.tile([HD, S], BF16, tag="q0_sb")
                q1_sb = homes.tile([HD, S], BF16, tag="q1_sb")
                kT_sb = homes.tile([HD, S], BF16, tag="kT_sb")
                vp_sb = homes.tile([128, KT, HD], BF16, tag="vp_sb")
                aT_sb = homes.tile([128, 2, S], F32R, tag="aT_sb")
                q_homes = [q0_sb, q1_sb]

                # ================= phase A: QKV projections + RoPE ========
                for c in range(NSC):
                    cs = slice(c * SC, (c + 1) * SC)
                    xc = xpool.tile([128, KT, SC], F32R, tag="xc")
                    nc.sync.dma_start(
                        out=xc,
                        in_=xT_d.ap()[b].rearrange("(k p) s -> p k s", p=128)[:, :, cs],
                    )
                    psq = ps.tile([128, 2 * SC], F32, tag="ps")
                    pskv = ps.tile([128, 2 * SC], F32, tag="ps")
                    for k in range(KT):
                        st = dict(start=(k == 0), stop=(k == KT - 1))
                        nc.tensor.matmul(psq[:, 0:SC], wq_sb[:, k, 0:HD], xc[:, k, :], **st)
                        nc.tensor.matmul(psq[:, SC:2 * SC], wq_sb[:, k, HD:2 * HD], xc[:, k, :], **st)
                    for k in range(KT):
                        st = dict(start=(k == 0), stop=(k == KT - 1))
                        nc.tensor.matmul(pskv[:, 0:SC], wk_sb[:, k, :], xc[:, k, :], **st)
                        nc.tensor.matmul(pskv[:, SC:2 * SC], wv_sb[:, k, :], xc[:, k, :], **st)

                    # raw q/k to SBUF bf16; rotate-half via PE matmul with the
                    # signed-permutation constant; rope combine on DVE
                    raws = []
                    for i, src in enumerate((psq[:, 0:SC], psq[:, SC:2 * SC],
                                             pskv[:, 0:SC])):
                        raw = stage.tile([128, SC], BF16, tag=f"raw{i}")
                        nc.scalar.copy(raw, src)
                        raws.append(raw)
                    psrot = ps.tile([128, 2 * SC], F32, tag="ps")
                    psrkv = ps.tile([128, SC], F32, tag="ps")
                    nc.tensor.matmul(psrot[:, 0:SC], rotm_sb, raws[0], start=True, stop=True)
                    nc.tensor.matmul(psrot[:, SC:2 * SC], rotm_sb, raws[1], start=True, stop=True)
                    nc.tensor.matmul(psrkv, rotm_sb, raws[2], start=True, stop=True)
                    # v: transpose to natural layout via PE (4x 128x128)
                    vt_sb = stage.tile([128, SC], BF16, tag="vt_sb")
                    nc.scalar.copy(vt_sb, pskv[:, SC:2 * SC])
                    pvt = ps.tile([128, SC], BF16, tag="ps")
                    for j in range(SC // 128):
                        nc.tensor.transpose(
                            pvt[:, j * 128:(j + 1) * 128],
                            vt_sb[:, j * 128:(j + 1) * 128],
                            iden_sb,
                        )
                    nc.vector.tensor_copy(
                        vp_sb[:, c * (SC // 128):(c + 1) * (SC // 128), :],
                        pvt,
                    )
                    for i, home in enumerate((q0_sb, q1_sb, kT_sb)):
                        rsrc = psrot[:, i * SC:(i + 1) * SC] if i < 2 else psrkv
                        tmp = stage.tile([128, SC], BF16, tag="tmp")
                        nc.vector.tensor_tensor(tmp, rsrc, sinrot_sb[:, cs], mult)
                        nc.vector.tensor_tensor(home[:, cs], raws[i], cos_sb[:, cs], mult)
                        nc.vector.tensor_tensor(home[:, cs], home[:, cs], tmp, add)

                # ================= phase B: attention ====================
                for h in range(2):
                    qh = q_homes[h]
                    for t in range(NQT):
                        qi0 = t * QT
                        nblk = (qi0 + QT) // 128
                        outT = ps.tile([128, QT], F32, tag="ps")
                        den = ps.tile([128, QT], F32, tag="ps")
                        seg_touchers = {}
                        for s0 in range(0, QT, SC):
                            js = [
                                j for j in range(nblk)
                                if max(j * 128 - qi0, 0) < s0 + SC
                            ]
                            seg_touchers[s0] = (js[0], js[-1])
                        for j in range(nblk):
                            kj0 = j * 128
                            r = kj0 - qi0
                            c0 = max(r, 0)
                            sT = ps.tile([128, QT], F32, tag="ps")
                            for s0 in range(0, QT, SC):
                                a0, a1 = max(c0, s0), s0 + SC
                                if a0 >= a1:
                                    continue
                                nc.tensor.matmul(
                                    sT[:, a0:a1],
                                    kT_sb[:, kj0:kj0 + 128],
                                    qh[:, qi0 + a0:qi0 + a1],
                                    start=True, stop=True,
                                )
                            if r >= 0:
                                nc.vector.tensor_tensor(
                                    sT[:, c0:c0 + 128], sT[:, c0:c0 + 128], mask_sb, add
                                )
                            pt = ptp.tile([128, QT], BF16, tag="pt")
                            nc.scalar.activation(
                                out=pt[:, c0:QT], in_=sT[:, c0:QT], func=AF.Exp, scale=SCALE
                            )
                            for s0 in range(0, QT, SC):
                                a0, a1 = max(c0, s0), s0 + SC
                                if a0 >= a1:
                                    continue
                                jf, jl = seg_touchers[s0]
                                st = dict(start=(j == jf), stop=(j == jl))
                                nc.tensor.matmul(
                                    outT[:, a0:a1], vp_sb[:, j, :], pt[:, a0:a1], **st
                                )
                                nc.tensor.matmul(
                                    den[:, a0:a1], ones_sb, pt[:, a0:a1], **st
                                )
                        # evacuate accumulators so the next tile's matmuls
                        # aren't blocked behind the slow reciprocal; chunk the
                        # reciprocal so it can't monopolize the DVE queue.
                        outT_sb = epi.tile([128, QT], F32, tag="outT_sb")
                        nc.scalar.copy(outT_sb, outT)
                        den_sb = epi.tile([128, QT], F32, tag="den_sb")
                        nc.scalar.copy(den_sb, den)
                        for e0 in range(0, QT, 256):
                            es = slice(e0, e0 + 256)
                            nc.vector.reciprocal(den_sb[:, es], den_sb[:, es])
                            nc.vector.tensor_tensor(
                                aT_sb[:, h, qi0 + e0:qi0 + e0 + 256],
                                outT_sb[:, es], den_sb[:, es], mult,
                            )

                # ================= phase C: o projection =================
                for m in range(S // 128):
                    ms = slice(m * 128, (m + 1) * 128)
                    for half in range(2):
                        pso = ps.tile([128, 1024], F32, tag="ps")
                        for ci in range(2):
                            for n0 in range(0, 1024, 512):
                                nc.tensor.matmul(
                                    pso[:, n0:n0 + 512],
                                    aT_sb[:, ci, ms],
                                    wo_sb[:, ci, half * 1024 + n0: half * 1024 + n0 + 512],
                                    start=(ci == 0), stop=(ci == 1),
                                )
                        os_sb = opool.tile([128, 1024], F32R, tag="os_sb")
                        if half == 0:
                            nc.vector.tensor_copy(os_sb, pso)
                        else:
                            nc.scalar.copy(os_sb, pso)
                        nc.sync.dma_start(
                            out=o_d.ap()[b, ms, half * 1024:(half + 1) * 1024],
                            in_=os_sb,
                        )

    legalize_waits(nc)
    return nc


_NC_CACHE = None


def _get_nc():
    global _NC_CACHE
    if _NC_CACHE is None:
        _NC_CACHE = build_nc()
    return _NC_CACHE


def _host_consts():
    inv = 1.0 / (ROPE_BASE ** (np.arange(0, HD, 2, dtype=np.float32) / HD))
    t = np.arange(S, dtype=np.float32)
    freqs = np.outer(t, inv)                       # [S, HD/2]
    emb = np.concatenate([freqs, freqs], axis=-1)  # [S, HD]
    cos = np.cos(emb)
    sin = np.sin(emb)
    cosT = np.ascontiguousarray(cos.T).astype(ml_dtypes.bfloat16)     # [HD, S]
    sinrotT = np.ascontiguousarray(sin.T).astype(np.float32)
    jj, ii = np.meshgrid(np.arange(128), np.arange(128), indexing="ij")
    addmask = np.where(jj <= ii, 0.0, -1e9).astype(np.float32)
    # rot(q)[d] = -q[d+64] (d<64), q[d-64] (d>=64); rot = R @ q and the PE
    # computes lhsT.T @ rhs, so pass R.T as the stationary operand.
    R = np.zeros((128, 128), dtype=np.float32)
    for d in range(64):
        R[d, d + 64] = -1.0
        R[d + 64, d] = 1.0
    rotmT = np.ascontiguousarray(R.T).astype(ml_dtypes.bfloat16)
    iden = np.eye(128, dtype=np.float32).astype(ml_dtypes.bfloat16)
    return cosT, sinrotT, addmask, rotmT, iden


def kernel(x, wq, wk, wv, wo):
    x = np.asarray(x, dtype=np.float32)
    wq = np.asarray(wq, dtype=np.float32)
    wk = np.asarray(wk, dtype=np.float32)
    wv = np.asarray(wv, dtype=np.float32)
    wo = np.asarray(wo, dtype=np.float32)

    xT = np.ascontiguousarray(x.transpose(0, 2, 1))  # [B, H, S]
    cosT, sinrotT, addmask, rotmT, iden = _host_consts()

    in_maps = []
    for c in range(N_CORES):
        g = c // 2  # kv head
        in_maps.append({
            "xT": xT,
            "wq": np.ascontiguousarray(wq[:, 2 * c * HD:(2 * c + 2) * HD]),
            "wk": np.ascontiguousarray(wk[:, g * HD:(g + 1) * HD]),
            "wv": np.ascontiguousarray(wv[:, g * HD:(g + 1) * HD]),
            "wo": np.ascontiguousarray(wo[2 * c * HD:(2 * c + 2) * HD, :]),
            "cosT": cosT,
            "sinrotT": sinrotT,
            "addmask": addmask,
            "rotmT": rotmT,
            "iden": iden,
        })

    nc = _get_nc()
    res = run_bass_kernel_spmd(nc, in_maps, core_ids=list(range(N_CORES)))
    globals()["_LAST_RESULT"] = res
    out = np.zeros((B, S, H), dtype=np.float64)
    for r in res.results:
        out += r["o"].astype(np.float64)
    return out.astype(np.float32)


if __name__ == "__main__":
    rng = np.random.default_rng(0)
    ins = {
        "x": rng.standard_normal((B, S, H), dtype=np.float32),
        "wq": rng.standard_normal((H, NH * HD), dtype=np.float32) * 0.02,
        "wk": rng.standard_normal((H, NKV * HD), dtype=np.float32) * 0.02,
        "wv": rng.standard_normal((H, NKV * HD), dtype=np.float32) * 0.02,
        "wo": rng.standard_normal((NH * HD, H), dtype=np.float32) * 0.02,
    }
    out = kernel(**ins)
    print("out", out.shape, out.dtype, float(np.abs(out).max()))

